# revision 21
# baseline (speedup 1.0000x reference)
"""DRQN fused kernel for 8 TRN2 NeuronCores.

Data-parallel over batch B=1024 -> 128 rows per core, L=6 timesteps.
Per core the whole net runs locally (no collectives):

    inp = concat(x, pos_onehot)      -- pos is constant (l0=0), so it is
                                        folded into the first-layer biases
    att = sigmoid(relu(inp@aw1+b)@aw2+b)       [768, 1]
    enc = relu(inp@ew1+b)@ew2+b                [768, 1024]
    out = cumsum_L(enc*att)                    [768, 1024]
    act = relu(relu(out@qw1+b)@qw2+b)@qw3+b    [768, 12972]

Dataflow keeps activations transposed in SBUF ([features, rows]) so every
weight matrix is the stationary matmul operand in its natural [K, M] layout
and the final action layer consumes H2^T directly, producing the output in
natural [rows, features] layout for contiguous DMA stores.  All matmuls run
as float32r (full-rate fp32 on the PE at free-dim >= 256).

Per-core roofline: 26.2 GFLOP -> ~337 us on the PE at 2.4 GHz; ~103 MB of
HBM traffic -> ~290 us at 358 GB/s.  Measured marginal exec time on HW is
~370-400 us; scale-relative error vs the fp32 reference is 3.6e-4.
"""

import numpy as np

import concourse.bass as bass
import concourse.mybir as mybir
from concourse import bacc
from concourse.bass_utils import run_bass_kernel_spmd
from concourse.masks import make_identity
from concourse.tile import TileContext

F32 = mybir.dt.float32
F32R = mybir.dt.float32r

L, B, N = 6, 1024, 512
G = 6
H, AH, ACT = 1024, 256, 12972
NCORES = 8
BL = B // NCORES          # 128 rows of batch per core
R = L * BL                # 768 rows per core
RG = 2                    # row groups
RGS = R // RG             # 384 rows per group (>=256 keeps f32r full rate)

# qw3 output tiling: 24x512 + 2x342 (all >= 256 for f32r full rate)
N_TILES = [512] * 24 + [342, 342]
N_OFFS = np.cumsum([0] + N_TILES)[:-1].tolist()

# tuning knobs (overridable before build_nc for experiments)
BUFS = dict(xn=6, tp=2, mm=4, att=2, ws=16, qw3s=32, pq=8, st=12, qb3=26)
STOP_AFTER = "full"  # enc1|enc2|q1|q2|full
POOL_MODE = "stack"  # or "queue"
WG = 2  # m-chunks per streamed weight tile for H x H layers
REPS = 1  # replicate whole computation in one NEFF (for benchmarking)


def r32(ap):
    return ap.bitcast(F32R)


def build_nc():
    nc = bacc.Bacc()

    x_h = nc.declare_dram_parameter("x", [L, BL, N], F32, isOutput=False)
    aw1_h = nc.declare_dram_parameter("aw1", [N, AH], F32R, isOutput=False)
    ab1_h = nc.declare_dram_parameter("ab1e", [AH], F32, isOutput=False)
    aw2_h = nc.declare_dram_parameter("aw2", [AH, 1], F32R, isOutput=False)
    ab2_h = nc.declare_dram_parameter("ab2", [1], F32, isOutput=False)
    ew1_h = nc.declare_dram_parameter("ew1", [N, H], F32R, isOutput=False)
    eb1_h = nc.declare_dram_parameter("eb1e", [H], F32, isOutput=False)
    ew2_h = nc.declare_dram_parameter("ew2", [H, H], F32R, isOutput=False)
    eb2_h = nc.declare_dram_parameter("eb2", [H], F32, isOutput=False)
    qw1_h = nc.declare_dram_parameter("qw1", [H, H], F32R, isOutput=False)
    qb1_h = nc.declare_dram_parameter("qb1", [H], F32, isOutput=False)
    qw2_h = nc.declare_dram_parameter("qw2", [H, H], F32R, isOutput=False)
    qb2_h = nc.declare_dram_parameter("qb2", [H], F32, isOutput=False)
    qw3_h = nc.declare_dram_parameter("qw3", [H, ACT], F32R, isOutput=False)
    qb3_h = nc.declare_dram_parameter("qb3", [ACT], F32R, isOutput=False)
    ones_h = nc.declare_dram_parameter("ones", [1, 128], F32R, isOutput=False)
    out_h = nc.declare_dram_parameter("out", [L, BL, ACT], F32, isOutput=True)

    KN = N // 128   # 4 K-chunks of x features
    KH = H // 128   # 8 chunks of hidden features
    KA = AH // 128  # 2 chunks of attention features
    MROW = R // 128  # 6 row chunks (row chunk m == timestep l)

    with TileContext(nc, pool_alloc_mode=POOL_MODE) as tc:
      for _rep in range(REPS):
        persist = tc.alloc_tile_pool(name="persist", bufs=1)
        ps_a = tc.alloc_tile_pool(name="ps_a", bufs=1, space="PSUM")
        pool_c = tc.alloc_tile_pool(name="pool_c", bufs=1)   # H1T + qw2 stream
        pool_b = tc.alloc_tile_pool(name="pool_b", bufs=1)   # E1T, GT, ew2/qw1 stream
        pool_1 = tc.alloc_tile_pool(name="pool_1", bufs=1)   # x, XT, aw1, ew1, A1T

        # ---- constants / biases ----
        ident = persist.tile([128, 128], F32, name="ident")
        make_identity(nc, ident)
        ones_t = persist.tile([1, 128], F32R, name="ones_t")
        nc.sync.dma_start(out=ones_t, in_=ones_h[:])

        ab1_t = persist.tile([128, KA], F32, name="ab1_t")
        nc.sync.dma_start(out=ab1_t, in_=ab1_h[:].rearrange("(m p) -> p m", p=128))
        eb1_t = persist.tile([128, KH], F32, name="eb1_t")
        nc.sync.dma_start(out=eb1_t, in_=eb1_h[:].rearrange("(m p) -> p m", p=128))
        eb2_t = persist.tile([128, KH], F32, name="eb2_t")
        nc.sync.dma_start(out=eb2_t, in_=eb2_h[:].rearrange("(m p) -> p m", p=128))
        qb1_t = persist.tile([128, KH], F32, name="qb1_t")
        nc.sync.dma_start(out=qb1_t, in_=qb1_h[:].rearrange("(m p) -> p m", p=128))
        qb2_t = persist.tile([128, KH], F32, name="qb2_t")
        nc.sync.dma_start(out=qb2_t, in_=qb2_h[:].rearrange("(m p) -> p m", p=128))
        ab2_t = persist.tile([1, 1], F32, name="ab2_t")
        nc.sync.dma_start(out=ab2_t, in_=ab2_h[:])

        att_s = persist.tile([1, R], F32R, name="att_s")
        att_bc = persist.tile([128, R], F32, name="att_bc")
        h2t = [persist.tile([128, R], F32R, name=f"h2t{k}", tag="h2t", bufs=KH)
               for k in range(KH)]

        # ---- load x and transpose to XT [feature, row] ----
        xt = [pool_1.tile([128, R], F32R, name=f"xt{k}", tag="xt", bufs=KN)
              for k in range(KN)]
        for l in range(L):
            xn = pool_1.tile([128, N], F32, name="xn", tag="xn", bufs=BUFS["xn"])
            nc.sync.dma_start(out=xn, in_=x_h[l])
            for k in range(KN):
                tp = ps_a.tile([128, 128], F32, name="tp", tag="tp", bufs=BUFS["tp"])
                nc.tensor.transpose(tp, xn[:, k * 128:(k + 1) * 128], ident)
                nc.scalar.copy(xt[k][:, l * 128:(l + 1) * 128], tp)

        # ---- attention branch ----
        aw1_t = [pool_1.tile([128, AH], F32R, name=f"aw1t{k}", tag="aw1t", bufs=KN)
                 for k in range(KN)]
        for k in range(KN):
            nc.sync.dma_start(out=aw1_t[k], in_=aw1_h[k * 128:(k + 1) * 128, :])
        aw2_t = [pool_1.tile([128, 1], F32R, name=f"aw2t{k}", tag="aw2t", bufs=KA)
                 for k in range(KA)]
        for k in range(KA):
            nc.sync.dma_start(out=aw2_t[k], in_=aw2_h[k * 128:(k + 1) * 128, :])

        a1t = [pool_1.tile([128, R], F32R, name=f"a1t{m}", tag="a1t", bufs=KA)
               for m in range(KA)]
        for m in range(KA):
            pls = [ps_a.tile([128, RGS], F32, name="a1ps", tag="mm", bufs=BUFS["mm"])
                   for _ in range(RG)]
            for k in range(KN):
                for g in range(RG):
                    nc.tensor.matmul(
                        pls[g], aw1_t[k][:, m * 128:(m + 1) * 128],
                        (xt[k][:, g * RGS:(g + 1) * RGS]),
                        start=(k == 0), stop=(k == KN - 1))
            for g in range(RG):
                nc.scalar.activation(
                    a1t[m][:, g * RGS:(g + 1) * RGS], pls[g],
                    mybir.ActivationFunctionType.Relu, bias=ab1_t[:, m:m + 1])

        for g in range(RG):
            aps = ps_a.tile([1, RGS], F32, name="aps", tag="att", bufs=BUFS["att"])
            for k in range(KA):
                nc.tensor.matmul(
                    aps, aw2_t[k], (a1t[k][:, g * RGS:(g + 1) * RGS]),
                    start=(k == 0), stop=(k == KA - 1))
            nc.scalar.activation(
                att_s[:, g * RGS:(g + 1) * RGS], aps,
                mybir.ActivationFunctionType.Sigmoid, bias=ab2_t[:, 0:1])
            bps = ps_a.tile([128, RGS], F32, name="bps", tag="att", bufs=BUFS["att"])
            nc.tensor.matmul(
                bps, (ones_t), (att_s[:, g * RGS:(g + 1) * RGS]),
                start=True, stop=True)
            nc.vector.tensor_copy(att_bc[:, g * RGS:(g + 1) * RGS], bps)

        # ---- encoder layer 1 ----
        ew1_t = [pool_1.tile([128, H], F32R, name=f"ew1t{k}", tag="ew1t", bufs=KN)
                 for k in range(KN)]
        for k in range(KN):
            nc.sync.dma_start(out=ew1_t[k], in_=ew1_h[k * 128:(k + 1) * 128, :])

        e1t = [pool_b.tile([128, R], F32R, name=f"e1t{m}", tag="e1t", bufs=KH)
               for m in range(KH)]
        for m in range(KH):
            pls = [ps_a.tile([128, RGS], F32, name="e1ps", tag="mm", bufs=BUFS["mm"])
                   for _ in range(RG)]
            for k in range(KN):
                for g in range(RG):
                    nc.tensor.matmul(
                        pls[g], ew1_t[k][:, m * 128:(m + 1) * 128],
                        (xt[k][:, g * RGS:(g + 1) * RGS]),
                        start=(k == 0), stop=(k == KN - 1))
            for g in range(RG):
                nc.scalar.activation(
                    e1t[m][:, g * RGS:(g + 1) * RGS], pls[g],
                    mybir.ActivationFunctionType.Relu, bias=eb1_t[:, m:m + 1])

        pool_1.release()

        _PH = ["enc1", "enc2", "q1", "q2", "full"].index(STOP_AFTER)

        # ---- encoder layer 2 + gate + cumsum over L ----
        gt = [pool_b.tile([128, R], F32R, name=f"gt{m}", tag="gt", bufs=KH)
              for m in range(KH)]
        for mg in (range(KH // WG) if _PH >= 1 else []):
            wts = [pool_b.tile([128, WG * 128], F32R, name="ew2s", tag="ew2s",
                               bufs=BUFS["ws"]) for _ in range(KH)]
            for k in range(KH):
                nc.sync.dma_start(
                    out=wts[k],
                    in_=ew2_h[k * 128:(k + 1) * 128,
                              mg * WG * 128:(mg + 1) * WG * 128])
            for mi in range(WG):
                m = mg * WG + mi
                pls = [ps_a.tile([128, RGS], F32, name="e2ps", tag="mm", bufs=BUFS["mm"])
                       for _ in range(RG)]
                for k in range(KH):
                    for g in range(RG):
                        nc.tensor.matmul(
                            pls[g], wts[k][:, mi * 128:(mi + 1) * 128],
                            (e1t[k][:, g * RGS:(g + 1) * RGS]),
                            start=(k == 0), stop=(k == KH - 1))
                for g in range(RG):
                    # gt = (psum + eb2) * att
                    nc.vector.scalar_tensor_tensor(
                        gt[m][:, g * RGS:(g + 1) * RGS], pls[g], eb2_t[:, m:m + 1],
                        att_bc[:, g * RGS:(g + 1) * RGS],
                        op0=mybir.AluOpType.add, op1=mybir.AluOpType.mult)
                # running sum over the 6 timesteps (128-row blocks of free dim)
                for l in range(1, L):
                    nc.vector.tensor_add(
                        gt[m][:, l * 128:(l + 1) * 128],
                        gt[m][:, l * 128:(l + 1) * 128],
                        gt[m][:, (l - 1) * 128:l * 128])

        # ---- q head layer 1 ----
        h1t = [pool_c.tile([128, R], F32R, name=f"h1t{m}", tag="h1t", bufs=KH)
               for m in range(KH)]
        for mg in (range(KH // WG) if _PH >= 2 else []):
            wts = [pool_b.tile([128, WG * 128], F32R, name="qw1s", tag="qw1s",
                               bufs=BUFS["ws"]) for _ in range(KH)]
            for k in range(KH):
                nc.sync.dma_start(
                    out=wts[k],
                    in_=qw1_h[k * 128:(k + 1) * 128,
                              mg * WG * 128:(mg + 1) * WG * 128])
            for mi in range(WG):
                m = mg * WG + mi
                pls = [ps_a.tile([128, RGS], F32, name="q1ps", tag="mm", bufs=BUFS["mm"])
                       for _ in range(RG)]
                for k in range(KH):
                    for g in range(RG):
                        nc.tensor.matmul(
                            pls[g], wts[k][:, mi * 128:(mi + 1) * 128],
                            (gt[k][:, g * RGS:(g + 1) * RGS]),
                            start=(k == 0), stop=(k == KH - 1))
                for g in range(RG):
                    nc.scalar.activation(
                        h1t[m][:, g * RGS:(g + 1) * RGS], pls[g],
                        mybir.ActivationFunctionType.Relu, bias=qb1_t[:, m:m + 1])

        pool_b.release()

        # ---- q head layer 2 ----
        for mg in (range(KH // WG) if _PH >= 3 else []):
            wts = [pool_c.tile([128, WG * 128], F32R, name="qw2s", tag="qw2s",
                               bufs=BUFS["ws"]) for _ in range(KH)]
            for k in range(KH):
                nc.sync.dma_start(
                    out=wts[k],
                    in_=qw2_h[k * 128:(k + 1) * 128,
                              mg * WG * 128:(mg + 1) * WG * 128])
            for mi in range(WG):
                m = mg * WG + mi
                pls = [ps_a.tile([128, RGS], F32, name="q2ps", tag="mm", bufs=BUFS["mm"])
                       for _ in range(RG)]
                for k in range(KH):
                    for g in range(RG):
                        nc.tensor.matmul(
                            pls[g], wts[k][:, mi * 128:(mi + 1) * 128],
                            (h1t[k][:, g * RGS:(g + 1) * RGS]),
                            start=(k == 0), stop=(k == KH - 1))
                for g in range(RG):
                    nc.scalar.activation(
                        h2t[m][:, g * RGS:(g + 1) * RGS], pls[g],
                        mybir.ActivationFunctionType.Relu, bias=qb2_t[:, m:m + 1])

        pool_c.release()
        ps_a.release()

        # ---- action layer: out[m*128+b, :] = h2 @ qw3 + qb3 ----
        pool_q3 = tc.alloc_tile_pool(name="pool_q3", bufs=1)
        ps_b = tc.alloc_tile_pool(name="ps_b", bufs=1, space="PSUM")

        qb3bc = []
        for nt, (n0, nn) in (enumerate(zip(N_OFFS, N_TILES)) if _PH >= 4 else []):
            qb3r = pool_q3.tile([1, 512], F32R, name="qb3r", tag="qb3r", bufs=2)
            nc.sync.dma_start(out=qb3r[:, :nn], in_=qb3_h[n0:n0 + nn][None, :])
            bps = ps_b.tile([128, 512], F32, name="q3bps", tag="pq", bufs=BUFS["pq"])
            nc.tensor.matmul(bps[:, :nn], ones_t, qb3r[:, :nn],
                             start=True, stop=True)
            t = pool_q3.tile([128, 512], F32, name=f"qb3bc{nt}", tag="qb3bc",
                             bufs=len(N_TILES))
            nc.vector.tensor_copy(t[:, :nn], bps[:, :nn])
            qb3bc.append(t)

        for nt, (n0, nn) in (enumerate(zip(N_OFFS, N_TILES)) if _PH >= 4 else []):
            wts = [pool_q3.tile([128, 512], F32R, name="qw3s", tag="qw3s", bufs=BUFS["qw3s"])
                   for _ in range(KH)]
            for k in range(KH):
                nc.sync.dma_start(out=wts[k][:, :nn],
                                  in_=qw3_h[k * 128:(k + 1) * 128, n0:n0 + nn])
            for m in range(MROW):
                pq = ps_b.tile([128, 512], F32, name="pq", tag="pq", bufs=BUFS["pq"])
                for k in range(KH):
                    nc.tensor.matmul(
                        pq[:, :nn], (h2t[k][:, m * 128:(m + 1) * 128]),
                        wts[k][:, :nn], start=(k == 0), stop=(k == KH - 1))
                st = pool_q3.tile([128, 512], F32, name="st", tag="st", bufs=BUFS["st"])
                nc.vector.tensor_add(st[:, :nn], pq[:, :nn], qb3bc[nt][:, :nn])
                nc.sync.dma_start(out=out_h[m, :, n0:n0 + nn], in_=st[:, :nn])

        pool_q3.release()
        ps_b.release()
        persist.release()

    nc.finalize()
    return nc


_NC_CACHE = {}
_BENCH_CACHE = {}


def _get_nc(reps=1):
    global REPS
    if reps not in _NC_CACHE:
        old = REPS
        REPS = reps
        try:
            _NC_CACHE[reps] = build_nc()
        finally:
            REPS = old
    return _NC_CACHE[reps]


def _prep_in_maps(inputs):
    f = lambda a: np.ascontiguousarray(np.asarray(a, dtype=np.float32))
    x = f(inputs["x"])
    aw1 = f(inputs["aw1"])
    ab1e = f(inputs["ab1"]) + aw1[N]          # fold pos one-hot (l0=0)
    ew1 = f(inputs["ew1"])
    eb1e = f(inputs["eb1"]) + ew1[N]
    shared = {
        "aw1": np.ascontiguousarray(aw1[:N]),
        "ab1e": ab1e,
        "aw2": f(inputs["aw2"]),
        "ab2": f(inputs["ab2"]),
        "ew1": np.ascontiguousarray(ew1[:N]),
        "eb1e": eb1e,
        "ew2": f(inputs["ew2"]),
        "eb2": f(inputs["eb2"]),
        "qw1": f(inputs["qw1"]),
        "qb1": f(inputs["qb1"]),
        "qw2": f(inputs["qw2"]),
        "qb2": f(inputs["qb2"]),
        "qw3": f(inputs["qw3"]),
        "qb3": f(inputs["qb3"]),
        "ones": np.ones((1, 128), dtype=np.float32),
    }
    in_maps = []
    for c in range(NCORES):
        m = dict(shared)
        m["x"] = np.ascontiguousarray(x[:, c * BL:(c + 1) * BL, :])
        in_maps.append(m)
    return in_maps


def run(inputs, **kwargs):
    import os
    os.environ.setdefault("BASS_NEVER_TRACE", "1")
    nc = _get_nc()
    in_maps = _prep_in_maps(inputs)
    res = run_bass_kernel_spmd(nc, in_maps, list(range(NCORES)), **kwargs)
    out = np.empty((L, B, ACT), dtype=np.float32)
    for c in range(NCORES):
        out[:, c * BL:(c + 1) * BL, :] = res.results[c]["out"]
    return out, res


def kernel(**inputs) -> np.ndarray:
    out, _ = run(inputs)
    return out


def bench(inputs, iters=20, warmup=3, reps=1):
    """Steady-state per-call wall time of the compiled 8-core NEFF with
    device-resident inputs (pipelined dispatch, single block at end)."""
    import time

    if reps in _BENCH_CACHE:
        sharded, concat_in, mk_zeros = _BENCH_CACHE[reps]
        for _ in range(warmup):
            out = sharded(*concat_in, *mk_zeros())
            import jax
            jax.block_until_ready(out)
        import jax
        zsets = [mk_zeros() for _ in range(iters)]
        jax.block_until_ready(zsets)
        t0 = time.perf_counter()
        outs = [sharded(*concat_in, *zsets[k]) for k in range(iters)]
        jax.block_until_ready(outs)
        t1 = time.perf_counter()
        return (t1 - t0) / iters * 1e9

    import jax
    from jax.sharding import Mesh, NamedSharding, PartitionSpec
    from jax.experimental.shard_map import shard_map

    import concourse.mybir as mybir_
    from concourse import bass2jax

    bass2jax.install_neuronx_cc_hook()
    nc = _get_nc(reps)
    in_maps = _prep_in_maps(inputs)

    partition_name = nc.partition_id_tensor.name if nc.partition_id_tensor else None
    in_names, out_names, out_avals, zero_shapes = [], [], [], []
    for alloc in nc.m.functions[0].allocations:
        if not isinstance(alloc, mybir_.MemoryLocationSet):
            continue
        name = alloc.memorylocations[0].name
        if alloc.kind == "ExternalInput":
            if name != partition_name:
                in_names.append(name)
        elif alloc.kind == "ExternalOutput":
            out_names.append(name)
            shape = tuple(alloc.tensor_shape)
            dtype = mybir_.dt.np(alloc.dtype)
            out_avals.append(jax.core.ShapedArray(shape, dtype))
            zero_shapes.append((shape, dtype))
    n_params = len(in_names)
    n_outs = len(out_avals)
    all_names = list(in_names) + list(out_names)
    if partition_name is not None:
        all_names.append(partition_name)
    donate = tuple(range(n_params, n_params + n_outs))

    def _body(*args):
        operands = list(args)
        if partition_name is not None:
            operands.append(bass2jax.partition_id_tensor())
        return tuple(bass2jax._bass_exec_p.bind(
            *operands,
            out_avals=tuple(out_avals),
            in_names=tuple(all_names),
            out_names=tuple(out_names),
            lowering_input_output_aliases=(),
            sim_require_finite=True,
            sim_require_nnan=True,
            nc=nc,
        ))

    devices = jax.devices()[:NCORES]
    mesh = Mesh(np.asarray(devices), ("core",))
    spec = NamedSharding(mesh, PartitionSpec("core"))
    in_specs = (PartitionSpec("core"),) * (n_params + n_outs)
    out_specs = (PartitionSpec("core"),) * n_outs
    sharded = jax.jit(
        shard_map(_body, mesh=mesh, in_specs=in_specs, out_specs=out_specs,
                  check_rep=False),
        donate_argnums=donate, keep_unused=True)

    concat_in = [
        jax.device_put(
            np.concatenate([np.asarray(in_maps[c][n]) for c in range(NCORES)],
                           axis=0), spec)
        for n in in_names
    ]

    def mk_zeros():
        import jax.numpy as jnp
        return [jax.device_put(
                    jnp.zeros((NCORES * s[0], *s[1:]), dt), spec)
                for (s, dt) in zero_shapes]

    _BENCH_CACHE[reps] = (sharded, concat_in, mk_zeros)

    for _ in range(warmup):
        out = sharded(*concat_in, *mk_zeros())
        jax.block_until_ready(out)

    zsets = [mk_zeros() for _ in range(iters)]
    jax.block_until_ready(zsets)
    t0 = time.perf_counter()
    outs = []
    for k in range(iters):
        outs.append(sharded(*concat_in, *zsets[k]))
    jax.block_until_ready(outs)
    t1 = time.perf_counter()
    pipelined_ns = (t1 - t0) / iters * 1e9

    # also per-call (sync) min
    times = []
    for _ in range(5):
        z = mk_zeros()
        jax.block_until_ready(z)
        t0 = time.perf_counter()
        o = sharded(*concat_in, *z)
        jax.block_until_ready(o)
        times.append(time.perf_counter() - t0)
    sync_min_ns = min(times) * 1e9
    print(f"bench: pipelined {pipelined_ns:.0f} ns/call, sync-min {sync_min_ns:.0f} ns")
    return pipelined_ns
